# revision 58
# baseline (speedup 1.0000x reference)
"""LIF spiking layer (T=32, B=256, C_in=C_out=4096, fp32) on 8 trn2 NeuronCores.

Strategy: data-parallel over batch (32 samples/core, W replicated).
Host-side numpy pre-permutes both operands into SBUF tile layout (contraction
dim ci on partitions), so each core only runs matmuls + the recurrence:
  current[co, (t,b)] = W @ x_core.T  on TensorE per 128-co tile (psum),
  LIF membrane recurrence over t on VectorE with mem laid out [co=128, b=32],
  spikes stored [co, (t,b)] and transposed back on the host.

Default MODE "fp16dr8" runs the matmul at 1.5 PE cycles per output column:
one fp16 hi-pass (1.0 cyc) plus ONE fp8-e4m3 DoubleRow pass (0.5 cyc) whose
two K-stacked groups carry BOTH hi/lo cross terms (Wl8@Xh8 + Wh8@Xl8). The
fp8 hi planes for X and W are derived on-chip from the fp16 tiles by
ScalarE/DVE, so the wire carries only fp16 + one fp8 residual plane each.
The first two co-tiles' matmuls interleave per k so the PE stays fed while
the X stream (DMA-bandwidth-bound) arrives, and the last two co-tiles run
as a pair with a joint recurrence over shrinking chunks so almost no serial
recurrence chain trails the final matmul. 672.8us cost-model time vs 1341us
for "fp16x3"; rel err 5.8e-3 (168 of 33.5M spikes flip, gate 2e-2).

MODE "fp32" is bit-exact vs the fp32 jax reference; "fp16x3" computes the
matmul as three fp16 hi/lo passes (~9e-4 rel err).
"""

import os

import ml_dtypes
import numpy as np

import concourse.mybir as mybir
import concourse.tile as tile
from concourse import bacc
from concourse.bass_utils import run_bass_kernel_spmd

FP32 = mybir.dt.float32
FP16 = mybir.dt.float16
FP8 = mybir.dt.float8e4

N_CORES = 8
T, B, CI, CO = 32, 256, 4096, 4096
B_LOC = B // N_CORES  # 32
TB = T * B_LOC  # 1024
# Exact powers of 2; the LIF recurrence is exactly scale-equivariant, and
# scaling keeps the fp16 lo-components out of subnormal range on the PE.
WSCALE = 64.0
XSCALE = 128.0
SCALE = WSCALE * XSCALE

# set by test.py to collect a profile
TRACE = False
LAST_EXEC_NS = None
# "fp16dr8": fp16 hi pass + ONE fp8-e4m3 DoubleRow pass that carries BOTH
# hi/lo cross terms K-stacked (Wl8@Xh8 + Wh8@Xl8 in a single 0.5-cyc/row
# matmul) — 1.5 PE cycles/row total vs fp16x3's 3.0. Host-emulated rel err
# 5.8e-3 (169 of 33.5M spikes flip).
# "fp16x3": 3-pass fp16 hi/lo split matmul — rel err ~9e-4 (4 of 33.5M spikes
# flip), ~1.35x faster than fp32, and robust across ~70 device runs.
# "fp32": bit-exact vs the fp32 reference (0 mismatches) but native-fp32
# matmul streams intermittently wedge the exec unit on this hardware
# (NRT_EXEC_UNIT_UNRECOVERABLE in 2 of 5 runs), so it is not the default.
MODE = os.environ.get("LIF_KERNEL_MODE", "fp16dr8")

# fp8 correction-pass scales: psum_corr = 2^CORR_E * (true correction in
# SCALE units). Both DoubleRow K-groups must land on the same 2^CORR_E.
WL8_S = 2.0**13  # Wl -> fp8  (alpha)
XH8_S = 2.0**-5  # Xh16 -> fp8 (beta);  alpha+beta = 8
WH8_S = 2.0**4   # Wh16 -> fp8 (gamma)
XL8_S = 2.0**4   # Xl -> fp8  (delta); gamma+delta = 8
CORR_E = 8

_CACHE = {}


def build_kernel_fp16dr8(
    d: float,
    th: float,
    has_bias: bool,
    T=T,
    B_loc=B_LOC,
    CI=CI,
    CO=CO,
):
    """fp16 hi pass + one K-stacked fp8 DoubleRow correction pass.

    psum_main[co, n] = sum_k Wh16[k, co] * Xh16[k, n]          (1.0 cyc/row)
    psum_corr[co, n] = sum_k Wl8[k,co]*Xh8[k,n] + Wh8[k,co]*Xl8[k,n]
                       (both terms in ONE DoubleRow matmul, 0.5 cyc/row)
    cur = psum_main + psum_corr * 2^-CORR_E   (combined into SBUF per chunk)
    then the LIF recurrence over t reads the SBUF staging tile.
    """
    TBl = T * B_loc
    n_k = CI // 128
    n_c = CO // 128
    csize = min(512, TBl)
    n_chunk = TBl // csize
    ths = float(th) * SCALE

    nc = bacc.Bacc("TRN2", target_bir_lowering=False, debug=False, num_devices=N_CORES)

    xh = nc.declare_dram_parameter("xh", [128, n_k, TBl], FP16, isOutput=False)
    xsl = nc.declare_dram_parameter("xsl", [128, n_k, TBl], FP8, isOutput=False)
    wh = nc.declare_dram_parameter("wh", [n_c, 128, n_k, 128], FP16, isOutput=False)
    wsl = nc.declare_dram_parameter("wsl", [n_c, 128, n_k, 128], FP8, isOutput=False)
    if has_bias:
        bias = nc.declare_dram_parameter("bias", [CO, 1], FP32, isOutput=False)
    spkT = nc.declare_dram_parameter("spkT", [CO, TBl], FP32, isOutput=True)
    # the last TWO c-tiles run as a pair with a joint recurrence; they stage
    # spikes in fp16 (exact for 0/1) to fit SBUF and the host widens them
    pair_tail = (not has_bias) and n_chunk == 2 and n_c >= 6
    if pair_tail:
        spkT_p = nc.declare_dram_parameter(
            "spkT_p", [128, 2, TBl], FP16, isOutput=True
        )

    with tile.TileContext(nc) as tc:
        with (
            tc.tile_pool(name="xt", bufs=1) as xt_pool,
            tc.tile_pool(name="wt", bufs=3) as wt_pool,
            tc.tile_pool(name="work", bufs=2) as work_pool,
            tc.tile_pool(name="pc", bufs=2 * n_chunk, space="PSUM") as pc_pool,
            tc.tile_pool(name="pcc", bufs=2 * n_chunk, space="PSUM") as pcc_pool,
        ):
            XH = xt_pool.tile([128, n_k, TBl], FP16)
            XS = xt_pool.tile([128, n_k, 2, TBl], FP8)
            sls = [slice(ch * csize, (ch + 1) * csize) for ch in range(n_chunk)]

            def load_w(c, WH_c, WS_c, ks=slice(None), conv="act"):
                """Wh fp16 + Wl fp8 from DRAM; Wh8 (group 1) derived on-chip
                from the fp16 tile, so only 1.5 MB/c hits the wire. During
                warmup the conversion runs on DVE (idle then) so ScalarE
                keeps up with the per-k X conversions."""
                nc.sync.dma_start(out=WH_c[:, ks, :], in_=wh[c, :, ks, :])
                nc.sync.dma_start(out=WS_c[:, ks, 0, :], in_=wsl[c, :, ks, :])
                if conv == "act":
                    nc.scalar.activation(
                        out=WS_c[:, ks, 1, :],
                        in_=WH_c[:, ks, :],
                        func=mybir.ActivationFunctionType.Copy,
                        scale=float(WH8_S),
                    )
                else:
                    nc.vector.tensor_scalar(
                        WS_c[:, ks, 1, :],
                        WH_c[:, ks, :],
                        float(WH8_S),
                        None,
                        mybir.AluOpType.mult,
                    )

            def load_x(ks):
                """Load a block of X k-tiles in two DMA transfers (HWDGE
                descriptor-gen is ~0.6us per transfer, so bigger blocks keep
                the warmup stream bandwidth-bound), then convert the fp8 hi
                plane per k so the PE's per-k deps stay fine-grained."""
                nc.sync.dma_start(out=XH[:, ks, :], in_=xh[:, ks, :])
                nc.sync.dma_start(out=XS[:, ks, 1, :], in_=xsl[:, ks, :])
                for k in range(ks.start, ks.stop):
                    nc.scalar.activation(
                        out=XS[:, k, 0, :],
                        in_=XH[:, k, :],
                        func=mybir.ActivationFunctionType.Copy,
                        scale=float(XH8_S),
                    )

            def alloc_w():
                return (
                    wt_pool.tile([128, n_k, 128], FP16, tag="wh", name="whc"),
                    wt_pool.tile([128, n_k, 2, 128], FP8, tag="ws", name="wsc"),
                )

            def alloc_psum():
                pms = [
                    pc_pool.tile([128, csize], FP32, tag="pm", name="pm")
                    for _ in range(n_chunk)
                ]
                pcrs = [
                    pcc_pool.tile([128, csize], FP32, tag="pcr", name="pcr")
                    for _ in range(n_chunk)
                ]
                return pms, pcrs

            def mm_main(pm, WH_c, k, sl):
                nc.tensor.matmul(
                    pm,
                    lhsT=WH_c[:, k, :],
                    rhs=XH[:, k, sl],
                    start=(k == 0),
                    stop=(k == n_k - 1),
                )

            def mm_corr(pcr, WS_c, k, sl):
                nc.tensor.matmul(
                    pcr,
                    lhsT=WS_c[:, k, :, :],
                    rhs=XS[:, k, :, sl],
                    start=(k == 0),
                    stop=(k == n_k - 1),
                    perf_mode=mybir.MatmulPerfMode.DoubleRow,
                )

            def combine(cur_sb, pm, pcr, sl):
                # cur = main + corr * 2^-CORR_E, staged to SBUF. A vector op
                # may read only ONE PSUM operand, so ScalarE copies the main
                # psum to SBUF and DVE adds the scaled correction.
                nc.scalar.activation(
                    out=cur_sb[:, sl],
                    in_=pm,
                    func=mybir.ActivationFunctionType.Copy,
                )
                nc.vector.scalar_tensor_tensor(
                    out=cur_sb[:, sl],
                    in0=pcr,
                    scalar=float(2.0**-CORR_E),
                    in1=cur_sb[:, sl],
                    op0=mybir.AluOpType.mult,
                    op1=mybir.AluOpType.add,
                )

            def alloc_mem():
                mem = work_pool.tile([128, B_loc], FP32, tag="mem", name="mem")
                s_stage = work_pool.tile([128, TBl], FP32, tag="s", name="s")
                nc.vector.memset(mem, 0.0)
                return mem, s_stage

            def recurrence(c, cur_sb, b_tile, mem_s, t0, t1, sl):
                """LIF recurrence for timesteps [t0, t1) of one chunk.
                Emitted per chunk (not per c-tile) so the DVE's in-order
                queue runs rec(ch0) before combine(ch1) and overlaps ch1's
                matmuls."""
                mem, s_stage = mem_s
                for t in range(t0, t1):
                    o = t * B_loc
                    nc.vector.scalar_tensor_tensor(
                        out=mem,
                        in0=mem,
                        scalar=d,
                        in1=cur_sb[:, o : o + B_loc],
                        op0=mybir.AluOpType.mult,
                        op1=mybir.AluOpType.add,
                    )
                    if has_bias:
                        nc.vector.tensor_scalar(
                            mem, mem, b_tile, None, mybir.AluOpType.add
                        )
                    s_t = s_stage[:, o : o + B_loc]
                    nc.vector.tensor_scalar(
                        s_t, mem, ths, None, mybir.AluOpType.is_gt
                    )
                    nc.vector.scalar_tensor_tensor(
                        out=mem,
                        in0=s_t,
                        scalar=-ths,
                        in1=mem,
                        op0=mybir.AluOpType.mult,
                        op1=mybir.AluOpType.add,
                    )
                # ship the finished chunk's spikes immediately so the final
                # c-tile's tail is one chunk, not the whole stage
                nc.sync.dma_start(
                    out=spkT[c * 128 : (c + 1) * 128, sl],
                    in_=s_stage[:, sl],
                )

            def load_bias(c):
                if not has_bias:
                    return None
                b_tile = work_pool.tile([128, 1], FP32, tag="bt", name="bt")
                nc.sync.dma_start(out=b_tile, in_=bias[c * 128 : (c + 1) * 128, :])
                return b_tile

            # ---- warmup stream: c0 AND c1 interleaved per k so the PE has
            # ~2.5us of matmul work per arriving X k-tile (the X stream is
            # DMA-bandwidth-paced). W tiles ride in a few big transfers
            # between X blocks; W2 prefetches right after the X bulk.
            WH0, WS0 = alloc_w()
            WH1, WS1 = alloc_w()
            wq = min(8, n_k)
            load_w(0, WH0, WS0, slice(0, wq), conv="dve")
            load_x(slice(0, 2))
            load_w(1, WH1, WS1, slice(0, wq), conv="dve")
            load_x(slice(2, 6))
            load_w(0, WH0, WS0, slice(8, 16), conv="dve")
            load_x(slice(6, 10))
            load_w(1, WH1, WS1, slice(8, 16), conv="dve")
            load_x(slice(10, 14))
            load_w(0, WH0, WS0, slice(16, 24), conv="dve")
            load_x(slice(14, 18))
            load_w(1, WH1, WS1, slice(16, 24), conv="dve")
            load_x(slice(18, 22))
            load_w(0, WH0, WS0, slice(24, n_k), conv="dve")
            load_x(slice(22, 26))
            load_w(1, WH1, WS1, slice(24, n_k), conv="dve")
            for k0 in range(26, n_k, 4):
                load_x(slice(k0, min(k0 + 4, n_k)))
            WH2, WS2 = alloc_w()
            load_w(2, WH2, WS2)

            b0 = load_bias(0)
            b1 = load_bias(1)
            cur0 = work_pool.tile([128, TBl], FP32, tag="cur", name="cur")
            cur1 = work_pool.tile([128, TBl], FP32, tag="cur", name="cur")
            pms0, pcrs0 = alloc_psum()
            pms1, pcrs1 = alloc_psum()
            t_pc = csize // B_loc
            for k in range(n_k):
                for pms, pcrs, WH_c, WS_c in (
                    (pms0, pcrs0, WH0, WS0),
                    (pms1, pcrs1, WH1, WS1),
                ):
                    for ch in range(n_chunk):
                        mm_main(pms[ch], WH_c, k, sls[ch])
                    for ch in range(n_chunk):
                        mm_corr(pcrs[ch], WS_c, k, sls[ch])
            ms0 = alloc_mem()
            ms1 = alloc_mem()
            for ch in range(n_chunk):
                combine(cur0, pms0[ch], pcrs0[ch], sls[ch])
                recurrence(0, cur0, b0, ms0, ch * t_pc, (ch + 1) * t_pc, sls[ch])
                combine(cur1, pms1[ch], pcrs1[ch], sls[ch])
                recurrence(1, cur1, b1, ms1, ch * t_pc, (ch + 1) * t_pc, sls[ch])

            # ---- steady state: chunk-sequential so each chunk's psum stops
            # early and the combine+recurrence overlap the next matmuls.
            # The last TWO c-tiles run as a pair with a joint recurrence
            # (ops on [128,2,B_loc] slices, half the serial chain per column)
            # over shrinking chunks, cutting the post-matmul tail to ~3us.
            steady_end = n_c - 2 if pair_tail else n_c
            for c in range(2, steady_end):
                if c == 2:
                    WH_c, WS_c = WH2, WS2
                else:
                    WH_c, WS_c = alloc_w()
                    load_w(c, WH_c, WS_c)
                b_tile = load_bias(c)
                cur_sb = work_pool.tile([128, TBl], FP32, tag="cur", name="cur")
                ms = alloc_mem()
                if c == n_c - 1 and csize >= 512:
                    bounds = list(range(0, TBl + 1, csize // 2))
                else:
                    bounds = list(range(0, TBl + 1, csize))
                for ch in range(len(bounds) - 1):
                    sl = slice(bounds[ch], bounds[ch + 1])
                    cs2 = bounds[ch + 1] - bounds[ch]
                    pm = pc_pool.tile([128, cs2], FP32, tag="pm", name="pm")
                    pcr = pcc_pool.tile([128, cs2], FP32, tag="pcr", name="pcr")
                    for k in range(n_k):
                        mm_main(pm, WH_c, k, sl)
                    for k in range(n_k):
                        mm_corr(pcr, WS_c, k, sl)
                    combine(cur_sb, pm, pcr, sl)
                    recurrence(
                        c,
                        cur_sb,
                        b_tile,
                        ms,
                        bounds[ch] // B_loc,
                        bounds[ch + 1] // B_loc,
                        sl,
                    )

            if pair_tail:
                cA, cB = n_c - 2, n_c - 1
                WHA, WSA = alloc_w()
                load_w(cA, WHA, WSA)
                WHB, WSB = alloc_w()
                load_w(cB, WHB, WSB)
                cur_p = work_pool.tile(
                    [128, 2, TBl], FP32, tag="curp", name="curp", bufs=1
                )
                s_p = work_pool.tile(
                    [128, 2, TBl], FP16, tag="sp", name="sp", bufs=1
                )
                mem_p = work_pool.tile(
                    [128, 2, B_loc], FP32, tag="memp", name="memp", bufs=1
                )
                nc.vector.memset(mem_p, 0.0)
                bounds = [0, 256, 512, 768, 896, 992, 1024]
                for ch in range(len(bounds) - 1):
                    sl = slice(bounds[ch], bounds[ch + 1])
                    cs2 = bounds[ch + 1] - bounds[ch]
                    pmA = pc_pool.tile([128, cs2], FP32, tag="pm", name="pm")
                    pcrA = pcc_pool.tile([128, cs2], FP32, tag="pcr", name="pcr")
                    pmB = pc_pool.tile([128, cs2], FP32, tag="pm", name="pm")
                    pcrB = pcc_pool.tile([128, cs2], FP32, tag="pcr", name="pcr")
                    for k in range(n_k):
                        mm_main(pmA, WHA, k, sl)
                    for k in range(n_k):
                        mm_corr(pcrA, WSA, k, sl)
                    for k in range(n_k):
                        mm_main(pmB, WHB, k, sl)
                    for k in range(n_k):
                        mm_corr(pcrB, WSB, k, sl)
                    for i, (pm, pcr) in enumerate(((pmA, pcrA), (pmB, pcrB))):
                        nc.scalar.activation(
                            out=cur_p[:, i, sl],
                            in_=pm,
                            func=mybir.ActivationFunctionType.Copy,
                        )
                        nc.vector.scalar_tensor_tensor(
                            out=cur_p[:, i, sl],
                            in0=pcr,
                            scalar=float(2.0**-CORR_E),
                            in1=cur_p[:, i, sl],
                            op0=mybir.AluOpType.mult,
                            op1=mybir.AluOpType.add,
                        )
                    for t in range(bounds[ch] // B_loc, bounds[ch + 1] // B_loc):
                        o = t * B_loc
                        nc.vector.scalar_tensor_tensor(
                            out=mem_p,
                            in0=mem_p,
                            scalar=d,
                            in1=cur_p[:, :, o : o + B_loc],
                            op0=mybir.AluOpType.mult,
                            op1=mybir.AluOpType.add,
                        )
                        s_t = s_p[:, :, o : o + B_loc]
                        nc.vector.tensor_scalar(
                            s_t, mem_p, ths, None, mybir.AluOpType.is_gt
                        )
                        nc.vector.scalar_tensor_tensor(
                            out=mem_p,
                            in0=s_t,
                            scalar=-ths,
                            in1=mem_p,
                            op0=mybir.AluOpType.mult,
                            op1=mybir.AluOpType.add,
                        )
                    nc.sync.dma_start(out=spkT_p[:, :, sl], in_=s_p[:, :, sl])

    nc.compile()
    return nc


def build_kernel_fp16x3(
    d: float,
    th: float,
    has_bias: bool,
    T=T,
    B_loc=B_LOC,
    CI=CI,
    CO=CO,
):
    """3-pass fp16 hi/lo kernel. All operands arrive from the host already
    split, scaled, and permuted into SBUF tile layout, so the device does
    only matmuls + the recurrence. Spikes leave in [co, tb] layout."""
    TBl = T * B_loc
    n_k = CI // 128
    n_c = CO // 128
    csize = min(512, TBl)
    n_chunk = TBl // csize
    ths = float(th) * SCALE

    nc = bacc.Bacc("TRN2", target_bir_lowering=False, debug=False, num_devices=N_CORES)

    xh = nc.declare_dram_parameter("xh", [128, n_k, TBl], FP16, isOutput=False)
    xl = nc.declare_dram_parameter("xl", [128, n_k, TBl], FP16, isOutput=False)
    wh = nc.declare_dram_parameter("wh", [n_c, 128, n_k, 128], FP16, isOutput=False)
    wl = nc.declare_dram_parameter("wl", [n_c, 128, n_k, 128], FP16, isOutput=False)
    if has_bias:
        bias = nc.declare_dram_parameter("bias", [CO, 1], FP32, isOutput=False)
    spkT = nc.declare_dram_parameter("spkT", [CO, TBl], FP32, isOutput=True)

    with tile.TileContext(nc) as tc:
        with (
            tc.tile_pool(name="xt", bufs=1) as xt_pool,
            tc.tile_pool(name="wt", bufs=2) as wt_pool,
            tc.tile_pool(name="work", bufs=2) as work_pool,
            tc.tile_pool(name="pc", bufs=2 * n_chunk, space="PSUM") as pc_pool,
        ):
            XH = xt_pool.tile([128, n_k, TBl], FP16)
            XL = xt_pool.tile([128, n_k, TBl], FP16)
            # first W strips ahead of the X bulk on the same HWDGE FIFO
            WH_first = wt_pool.tile([128, n_k, 128], FP16, tag="wh")
            WL_first = wt_pool.tile([128, n_k, 128], FP16, tag="wl")
            wq = min(8, n_k)
            for kq in range(0, n_k, wq):
                nc.sync.dma_start(
                    out=WH_first[:, kq : kq + wq, :], in_=wh[0, :, kq : kq + wq, :]
                )
            nc.sync.dma_start(out=WL_first, in_=wl[0, :, :, :])
            for k in range(n_k):
                nc.sync.dma_start(out=XH[:, k, :], in_=xh[:, k, :])
                nc.sync.dma_start(out=XL[:, k, :], in_=xl[:, k, :])

            for c in range(n_c):
                if c == 0:
                    WH_c, WL_c = WH_first, WL_first
                else:
                    WH_c = wt_pool.tile([128, n_k, 128], FP16, tag="wh")
                    WL_c = wt_pool.tile([128, n_k, 128], FP16, tag="wl")
                    nc.sync.dma_start(out=WH_c, in_=wh[c, :, :, :])
                    nc.sync.dma_start(out=WL_c, in_=wl[c, :, :, :])
                if has_bias:
                    b_tile = work_pool.tile([128, 1], FP32, tag="bt")
                    nc.sync.dma_start(
                        out=b_tile, in_=bias[c * 128 : (c + 1) * 128, :]
                    )

                pcs = [
                    pc_pool.tile([128, csize], FP32, tag="pc", name="pc")
                    for _ in range(n_chunk)
                ]
                n_mm = 3 * n_k
                if c == 0:
                    # consume in DMA arrival order: all passes of k before k+1
                    order = [(k, p) for k in range(n_k) for p in (0, 1, 2)]
                else:
                    order = [(k, p) for p in (0, 1, 2) for k in range(n_k)]
                for ch in range(n_chunk):
                    ops = ((WH_c, XH), (WL_c, XH), (WH_c, XL))
                    for i, (k, p) in enumerate(order):
                        Wt, Xt = ops[p]
                        nc.tensor.matmul(
                            pcs[ch],
                            lhsT=Wt[:, k, :],
                            rhs=Xt[:, k, ch * csize : (ch + 1) * csize],
                            start=(i == 0),
                            stop=(i == n_mm - 1),
                        )

                mem = work_pool.tile([128, B_loc], FP32, tag="mem")
                s_stage = work_pool.tile([128, TBl], FP32, tag="s")
                nc.vector.memset(mem, 0.0)
                for t in range(T):
                    o = t * B_loc
                    cur = pcs[o // csize][:, o % csize : o % csize + B_loc]
                    nc.vector.scalar_tensor_tensor(
                        out=mem,
                        in0=mem,
                        scalar=d,
                        in1=cur,
                        op0=mybir.AluOpType.mult,
                        op1=mybir.AluOpType.add,
                    )
                    if has_bias:
                        nc.vector.tensor_scalar(
                            mem, mem, b_tile, None, mybir.AluOpType.add
                        )
                    s_t = s_stage[:, o : o + B_loc]
                    nc.vector.tensor_scalar(
                        s_t, mem, ths, None, mybir.AluOpType.is_gt
                    )
                    nc.vector.scalar_tensor_tensor(
                        out=mem,
                        in0=s_t,
                        scalar=-ths,
                        in1=mem,
                        op0=mybir.AluOpType.mult,
                        op1=mybir.AluOpType.add,
                    )

                nc.sync.dma_start(
                    out=spkT[c * 128 : (c + 1) * 128, :], in_=s_stage
                )

    nc.compile()
    return nc


def build_kernel_fp32hp(
    d: float,
    th: float,
    has_bias: bool,
    T=T,
    B_loc=B_LOC,
    CI=CI,
    CO=CO,
):
    """Exact-fp32 kernel with host-prepped transposed layouts: the device does
    only fp32 matmuls + the recurrence. Spikes leave in [co, tb] layout."""
    TBl = T * B_loc
    n_k = CI // 128
    n_c = CO // 128
    csize = min(512, TBl)
    n_chunk = TBl // csize

    nc = bacc.Bacc("TRN2", target_bir_lowering=False, debug=False, num_devices=N_CORES)

    xt = nc.declare_dram_parameter("xt", [128, n_k, TBl], FP32, isOutput=False)
    wt = nc.declare_dram_parameter("wt", [n_c, 128, n_k, 128], FP32, isOutput=False)
    if has_bias:
        bias = nc.declare_dram_parameter("bias", [CO, 1], FP32, isOutput=False)
    spkT = nc.declare_dram_parameter("spkT", [CO, TBl], FP32, isOutput=True)

    with tile.TileContext(nc) as tc:
        with (
            tc.tile_pool(name="xtp", bufs=1) as xt_pool,
            tc.tile_pool(name="wtp", bufs=3) as wt_pool,
            tc.tile_pool(name="work", bufs=2) as work_pool,
            tc.tile_pool(name="pc", bufs=4 * n_chunk, space="PSUM") as pc_pool,
        ):
            XT = xt_pool.tile([128, n_k, TBl], FP32)
            # first W strip ahead of the XT bulk on the same HWDGE FIFO, in
            # k-chunks, so co-tile 0's first matmuls start almost immediately
            WT_first = wt_pool.tile([128, n_k, 128], FP32, tag="wt")
            wq = min(8, n_k)
            for kq in range(0, n_k, wq):
                nc.sync.dma_start(
                    out=WT_first[:, kq : kq + wq, :], in_=wt[0, :, kq : kq + wq, :]
                )
            # per-k loads so co-tile 0 consumes tiles in DMA arrival order
            for k in range(n_k):
                nc.sync.dma_start(out=XT[:, k, :], in_=xt[:, k, :])

            for c in range(n_c):
                if c == 0:
                    WT_c = WT_first
                else:
                    WT_c = wt_pool.tile([128, n_k, 128], FP32, tag="wt")
                    nc.sync.dma_start(out=WT_c, in_=wt[c, :, :, :])
                if has_bias:
                    b_tile = work_pool.tile([128, 1], FP32, tag="bt")
                    nc.sync.dma_start(
                        out=b_tile, in_=bias[c * 128 : (c + 1) * 128, :]
                    )

                pcs = [
                    pc_pool.tile([128, csize], FP32, tag="pc", name="pc")
                    for _ in range(n_chunk)
                ]
                if c == 0:
                    # k outer: consume XT tiles as they arrive from DRAM
                    for k in range(n_k):
                        for ch in range(n_chunk):
                            nc.tensor.matmul(
                                pcs[ch],
                                lhsT=WT_c[:, k, :],
                                rhs=XT[:, k, ch * csize : (ch + 1) * csize],
                                start=(k == 0),
                                stop=(k == n_k - 1),
                            )
                else:
                    # chunk outer: chunk0 psum frees early for the recurrence
                    for ch in range(n_chunk):
                        for k in range(n_k):
                            nc.tensor.matmul(
                                pcs[ch],
                                lhsT=WT_c[:, k, :],
                                rhs=XT[:, k, ch * csize : (ch + 1) * csize],
                                start=(k == 0),
                                stop=(k == n_k - 1),
                            )

                mem = work_pool.tile([128, B_loc], FP32, tag="mem")
                s_stage = work_pool.tile([128, TBl], FP32, tag="s")
                nc.vector.memset(mem, 0.0)
                for t in range(T):
                    o = t * B_loc
                    cur = pcs[o // csize][:, o % csize : o % csize + B_loc]
                    nc.vector.scalar_tensor_tensor(
                        out=mem,
                        in0=mem,
                        scalar=d,
                        in1=cur,
                        op0=mybir.AluOpType.mult,
                        op1=mybir.AluOpType.add,
                    )
                    if has_bias:
                        nc.vector.tensor_scalar(
                            mem, mem, b_tile, None, mybir.AluOpType.add
                        )
                    s_t = s_stage[:, o : o + B_loc]
                    nc.vector.tensor_scalar(
                        s_t, mem, float(th), None, mybir.AluOpType.is_gt
                    )
                    nc.vector.scalar_tensor_tensor(
                        out=mem,
                        in0=s_t,
                        scalar=-float(th),
                        in1=mem,
                        op0=mybir.AluOpType.mult,
                        op1=mybir.AluOpType.add,
                    )

                nc.sync.dma_start(
                    out=spkT[c * 128 : (c + 1) * 128, :], in_=s_stage
                )

    nc.compile()
    return nc


def _f8(a32, scale):
    """fp32 -> float8_e4m3 at the given power-of-two scale, clipped to range."""
    return np.clip(a32 * np.float32(scale), -240.0, 240.0).astype(
        ml_dtypes.float8_e4m3
    )


def _split16(a32):
    hi = a32.astype(np.float16)
    lo = (a32 - hi.astype(np.float32)).astype(np.float16)
    return hi, lo


def _xt_layout(xs):
    """[TB, CI] -> [128, CI//128, TB] so SBUF partition p holds ci = k*128+p."""
    TBl, CIl = xs.shape
    return np.ascontiguousarray(
        xs.reshape(TBl, CIl // 128, 128).transpose(2, 1, 0)
    )


def _wt_layout(Wm):
    """[CO, CI] -> [CO//128, 128, CI//128, 128]: strip c, partition p=ci%128,
    k=ci//128, j=co%128 -> W[c*128+j, k*128+p]."""
    COl, CIl = Wm.shape
    return np.ascontiguousarray(
        Wm.reshape(COl // 128, 128, CIl // 128, 128).transpose(0, 3, 2, 1)
    )


def kernel(x, W, b, decay, thresh):
    global LAST_EXEC_NS
    x = np.ascontiguousarray(np.asarray(x, dtype=np.float32))
    W = np.ascontiguousarray(np.asarray(W, dtype=np.float32))
    b = np.asarray(b, dtype=np.float32)
    decay = np.asarray(decay, dtype=np.float32)
    thresh = np.asarray(thresh, dtype=np.float32)

    d = float(decay.reshape(-1)[0])
    th = float(thresh.reshape(-1)[0])
    has_bias = bool(np.any(b != 0))

    key = (MODE, d, th, has_bias)
    if key not in _CACHE:
        if MODE == "fp16dr8":
            _CACHE[key] = build_kernel_fp16dr8(d, th, has_bias)
        elif MODE == "fp16x3":
            _CACHE[key] = build_kernel_fp16x3(d, th, has_bias)
        else:
            _CACHE[key] = build_kernel_fp32hp(d, th, has_bias)
    nc = _CACHE[key]

    in_maps = []
    if MODE == "fp16dr8":
        W64 = W * np.float32(WSCALE)
        Wh16 = W64.astype(np.float16)
        Wl = W64 - Wh16.astype(np.float32)
        wh_l = _wt_layout(Wh16)
        wsl_l = _wt_layout(_f8(Wl, WL8_S))
        for i in range(N_CORES):
            xs_i = x[:, i * B_LOC : (i + 1) * B_LOC, :].reshape(TB, CI)
            X128 = xs_i * np.float32(XSCALE)
            Xh16 = X128.astype(np.float16)
            Xl = X128 - Xh16.astype(np.float32)
            m = {
                "xh": _xt_layout(Xh16),
                "xsl": _xt_layout(_f8(Xl, XL8_S)),
                "wh": wh_l,
                "wsl": wsl_l,
            }
            if has_bias:
                m["bias"] = np.ascontiguousarray(
                    (b * np.float32(SCALE)).reshape(CO, 1)
                )
            in_maps.append(m)
    elif MODE == "fp16x3":
        Wh, Wl = _split16(W * np.float32(WSCALE))
        wh_l = _wt_layout(Wh)
        wl_l = _wt_layout(Wl)
        for i in range(N_CORES):
            xs_i = x[:, i * B_LOC : (i + 1) * B_LOC, :].reshape(TB, CI)
            xh_i, xl_i = _split16(xs_i * np.float32(XSCALE))
            m = {
                "xh": _xt_layout(xh_i),
                "xl": _xt_layout(xl_i),
                "wh": wh_l,
                "wl": wl_l,
            }
            if has_bias:
                m["bias"] = np.ascontiguousarray(
                    (b * np.float32(SCALE)).reshape(CO, 1)
                )
            in_maps.append(m)
    else:
        wt_l = _wt_layout(W)
        for i in range(N_CORES):
            xs_i = x[:, i * B_LOC : (i + 1) * B_LOC, :].reshape(TB, CI)
            m = {"xt": _xt_layout(xs_i), "wt": wt_l}
            if has_bias:
                m["bias"] = np.ascontiguousarray(b.reshape(CO, 1))
            in_maps.append(m)

    res = run_bass_kernel_spmd(
        nc, in_maps, core_ids=list(range(N_CORES)), trace=TRACE
    )
    LAST_EXEC_NS = res.exec_time_ns

    # spikes come back [CO, TB]; transpose to [T, B_loc, CO] per core. The
    # paired last two c-tiles return via the fp16 spkT_p plane.
    def unshard(r):
        spk = r["spkT"]
        if "spkT_p" in r:
            spk = spk.copy()
            # spkT_p is [128, 2, TB]: group i holds c-tile (n_c-2+i)'s rows
            spk[CO - 256 : CO - 128, :] = r["spkT_p"][:, 0, :].astype(np.float32)
            spk[CO - 128 :, :] = r["spkT_p"][:, 1, :].astype(np.float32)
        return np.ascontiguousarray(spk.T).reshape(T, B_LOC, CO)

    out = np.concatenate([unshard(r) for r in res.results], axis=1)
    return np.ascontiguousarray(out)



# revision 60
# speedup vs baseline: 1.0003x; 1.0003x over previous
"""LIF spiking layer (T=32, B=256, C_in=C_out=4096, fp32) on 8 trn2 NeuronCores.

Strategy: data-parallel over batch (32 samples/core, W replicated).
Host-side numpy pre-permutes both operands into SBUF tile layout (contraction
dim ci on partitions), so each core only runs matmuls + the recurrence:
  current[co, (t,b)] = W @ x_core.T  on TensorE per 128-co tile (psum),
  LIF membrane recurrence over t on VectorE with mem laid out [co=128, b=32],
  spikes stored [co, (t,b)] and transposed back on the host.

Default MODE "fp16dr8" runs the matmul at 1.5 PE cycles per output column:
one fp16 hi-pass (1.0 cyc) plus ONE fp8-e4m3 DoubleRow pass (0.5 cyc) whose
two K-stacked groups carry BOTH hi/lo cross terms (Wl8@Xh8 + Wh8@Xl8). The
fp8 hi planes for X and W are derived on-chip from the fp16 tiles by
ScalarE/DVE, so the wire carries only fp16 + one fp8 residual plane each.
The first two co-tiles' matmuls interleave per k so the PE stays fed while
the X stream (DMA-bandwidth-bound) arrives, and the last two co-tiles run
as a pair with a joint recurrence over shrinking chunks so almost no serial
recurrence chain trails the final matmul. 672.8us cost-model time vs 1341us
for "fp16x3"; rel err 5.8e-3 (168 of 33.5M spikes flip, gate 2e-2).

MODE "fp32" is bit-exact vs the fp32 jax reference; "fp16x3" computes the
matmul as three fp16 hi/lo passes (~9e-4 rel err).
"""

import os

import ml_dtypes
import numpy as np

import concourse.mybir as mybir
import concourse.tile as tile
from concourse import bacc
from concourse.bass_utils import run_bass_kernel_spmd

FP32 = mybir.dt.float32
FP16 = mybir.dt.float16
FP8 = mybir.dt.float8e4

N_CORES = 8
T, B, CI, CO = 32, 256, 4096, 4096
B_LOC = B // N_CORES  # 32
TB = T * B_LOC  # 1024
# Exact powers of 2; the LIF recurrence is exactly scale-equivariant, and
# scaling keeps the fp16 lo-components out of subnormal range on the PE.
WSCALE = 64.0
XSCALE = 128.0
SCALE = WSCALE * XSCALE

# set by test.py to collect a profile
TRACE = False
LAST_EXEC_NS = None
# "fp16dr8": fp16 hi pass + ONE fp8-e4m3 DoubleRow pass that carries BOTH
# hi/lo cross terms K-stacked (Wl8@Xh8 + Wh8@Xl8 in a single 0.5-cyc/row
# matmul) — 1.5 PE cycles/row total vs fp16x3's 3.0. Host-emulated rel err
# 5.8e-3 (169 of 33.5M spikes flip).
# "fp16x3": 3-pass fp16 hi/lo split matmul — rel err ~9e-4 (4 of 33.5M spikes
# flip), ~1.35x faster than fp32, and robust across ~70 device runs.
# "fp32": bit-exact vs the fp32 reference (0 mismatches) but native-fp32
# matmul streams intermittently wedge the exec unit on this hardware
# (NRT_EXEC_UNIT_UNRECOVERABLE in 2 of 5 runs), so it is not the default.
MODE = os.environ.get("LIF_KERNEL_MODE", "fp16dr8")

# fp8 correction-pass scales: psum_corr = 2^CORR_E * (true correction in
# SCALE units). Both DoubleRow K-groups must land on the same 2^CORR_E.
WL8_S = 2.0**13  # Wl -> fp8  (alpha)
XH8_S = 2.0**-5  # Xh16 -> fp8 (beta);  alpha+beta = 8
WH8_S = 2.0**4   # Wh16 -> fp8 (gamma)
XL8_S = 2.0**4   # Xl -> fp8  (delta); gamma+delta = 8
CORR_E = 8

_CACHE = {}


def build_kernel_fp16dr8(
    d: float,
    th: float,
    has_bias: bool,
    T=T,
    B_loc=B_LOC,
    CI=CI,
    CO=CO,
):
    """fp16 hi pass + one K-stacked fp8 DoubleRow correction pass.

    psum_main[co, n] = sum_k Wh16[k, co] * Xh16[k, n]          (1.0 cyc/row)
    psum_corr[co, n] = sum_k Wl8[k,co]*Xh8[k,n] + Wh8[k,co]*Xl8[k,n]
                       (both terms in ONE DoubleRow matmul, 0.5 cyc/row)
    cur = psum_main + psum_corr * 2^-CORR_E   (combined into SBUF per chunk)
    then the LIF recurrence over t reads the SBUF staging tile.
    """
    TBl = T * B_loc
    n_k = CI // 128
    n_c = CO // 128
    csize = min(512, TBl)
    n_chunk = TBl // csize
    ths = float(th) * SCALE

    nc = bacc.Bacc("TRN2", target_bir_lowering=False, debug=False, num_devices=N_CORES)

    xh = nc.declare_dram_parameter("xh", [128, n_k, TBl], FP16, isOutput=False)
    xsl = nc.declare_dram_parameter("xsl", [128, n_k, TBl], FP8, isOutput=False)
    wh = nc.declare_dram_parameter("wh", [n_c, 128, n_k, 128], FP16, isOutput=False)
    wsl = nc.declare_dram_parameter("wsl", [n_c, 128, n_k, 128], FP8, isOutput=False)
    if has_bias:
        bias = nc.declare_dram_parameter("bias", [CO, 1], FP32, isOutput=False)
    spkT = nc.declare_dram_parameter("spkT", [CO, TBl], FP32, isOutput=True)
    # the last TWO c-tiles run as a pair with a joint recurrence; they stage
    # spikes in fp16 (exact for 0/1) to fit SBUF and the host widens them
    pair_tail = (not has_bias) and n_chunk == 2 and n_c >= 6
    if pair_tail:
        spkT_p = nc.declare_dram_parameter(
            "spkT_p", [128, 2, TBl], FP16, isOutput=True
        )

    with tile.TileContext(nc) as tc:
        with (
            tc.tile_pool(name="xt", bufs=1) as xt_pool,
            tc.tile_pool(name="wt", bufs=3) as wt_pool,
            tc.tile_pool(name="work", bufs=2) as work_pool,
            tc.tile_pool(name="pc", bufs=2 * n_chunk, space="PSUM") as pc_pool,
            tc.tile_pool(name="pcc", bufs=2 * n_chunk, space="PSUM") as pcc_pool,
        ):
            XH = xt_pool.tile([128, n_k, TBl], FP16)
            XS = xt_pool.tile([128, n_k, 2, TBl], FP8)
            sls = [slice(ch * csize, (ch + 1) * csize) for ch in range(n_chunk)]

            def load_w(c, WH_c, WS_c, ks=slice(None), conv="act"):
                """Wh fp16 + Wl fp8 from DRAM; Wh8 (group 1) derived on-chip
                from the fp16 tile, so only 1.5 MB/c hits the wire. During
                warmup the conversion runs on DVE (idle then) so ScalarE
                keeps up with the per-k X conversions."""
                nc.sync.dma_start(out=WH_c[:, ks, :], in_=wh[c, :, ks, :])
                nc.sync.dma_start(out=WS_c[:, ks, 0, :], in_=wsl[c, :, ks, :])
                if conv == "act":
                    nc.scalar.activation(
                        out=WS_c[:, ks, 1, :],
                        in_=WH_c[:, ks, :],
                        func=mybir.ActivationFunctionType.Copy,
                        scale=float(WH8_S),
                    )
                else:
                    nc.vector.tensor_scalar(
                        WS_c[:, ks, 1, :],
                        WH_c[:, ks, :],
                        float(WH8_S),
                        None,
                        mybir.AluOpType.mult,
                    )

            def load_x(ks):
                """Load a block of X k-tiles in two DMA transfers (HWDGE
                descriptor-gen is ~0.6us per transfer, so bigger blocks keep
                the warmup stream bandwidth-bound), then convert the fp8 hi
                plane per k so the PE's per-k deps stay fine-grained."""
                nc.sync.dma_start(out=XH[:, ks, :], in_=xh[:, ks, :])
                nc.sync.dma_start(out=XS[:, ks, 1, :], in_=xsl[:, ks, :])
                for k in range(ks.start, ks.stop):
                    nc.scalar.activation(
                        out=XS[:, k, 0, :],
                        in_=XH[:, k, :],
                        func=mybir.ActivationFunctionType.Copy,
                        scale=float(XH8_S),
                    )

            def alloc_w():
                return (
                    wt_pool.tile([128, n_k, 128], FP16, tag="wh", name="whc"),
                    wt_pool.tile([128, n_k, 2, 128], FP8, tag="ws", name="wsc"),
                )

            def alloc_psum():
                pms = [
                    pc_pool.tile([128, csize], FP32, tag="pm", name="pm")
                    for _ in range(n_chunk)
                ]
                pcrs = [
                    pcc_pool.tile([128, csize], FP32, tag="pcr", name="pcr")
                    for _ in range(n_chunk)
                ]
                return pms, pcrs

            def mm_main(pm, WH_c, k, sl):
                nc.tensor.matmul(
                    pm,
                    lhsT=WH_c[:, k, :],
                    rhs=XH[:, k, sl],
                    start=(k == 0),
                    stop=(k == n_k - 1),
                )

            def mm_corr(pcr, WS_c, k, sl):
                nc.tensor.matmul(
                    pcr,
                    lhsT=WS_c[:, k, :, :],
                    rhs=XS[:, k, :, sl],
                    start=(k == 0),
                    stop=(k == n_k - 1),
                    perf_mode=mybir.MatmulPerfMode.DoubleRow,
                )

            def combine(cur_sb, pm, pcr, sl):
                # cur = main + corr * 2^-CORR_E, staged to SBUF. A vector op
                # may read only ONE PSUM operand, so ScalarE copies the main
                # psum to SBUF and DVE adds the scaled correction.
                nc.scalar.activation(
                    out=cur_sb[:, sl],
                    in_=pm,
                    func=mybir.ActivationFunctionType.Copy,
                )
                nc.vector.scalar_tensor_tensor(
                    out=cur_sb[:, sl],
                    in0=pcr,
                    scalar=float(2.0**-CORR_E),
                    in1=cur_sb[:, sl],
                    op0=mybir.AluOpType.mult,
                    op1=mybir.AluOpType.add,
                )

            def alloc_mem():
                mem = work_pool.tile([128, B_loc], FP32, tag="mem", name="mem")
                s_stage = work_pool.tile([128, TBl], FP32, tag="s", name="s")
                nc.vector.memset(mem, 0.0)
                return mem, s_stage

            def recurrence(c, cur_sb, b_tile, mem_s, t0, t1, sl):
                """LIF recurrence for timesteps [t0, t1) of one chunk.
                Emitted per chunk (not per c-tile) so the DVE's in-order
                queue runs rec(ch0) before combine(ch1) and overlaps ch1's
                matmuls."""
                mem, s_stage = mem_s
                for t in range(t0, t1):
                    o = t * B_loc
                    nc.vector.scalar_tensor_tensor(
                        out=mem,
                        in0=mem,
                        scalar=d,
                        in1=cur_sb[:, o : o + B_loc],
                        op0=mybir.AluOpType.mult,
                        op1=mybir.AluOpType.add,
                    )
                    if has_bias:
                        nc.vector.tensor_scalar(
                            mem, mem, b_tile, None, mybir.AluOpType.add
                        )
                    s_t = s_stage[:, o : o + B_loc]
                    nc.vector.tensor_scalar(
                        s_t, mem, ths, None, mybir.AluOpType.is_gt
                    )
                    nc.vector.scalar_tensor_tensor(
                        out=mem,
                        in0=s_t,
                        scalar=-ths,
                        in1=mem,
                        op0=mybir.AluOpType.mult,
                        op1=mybir.AluOpType.add,
                    )
                # ship the finished chunk's spikes immediately so the final
                # c-tile's tail is one chunk, not the whole stage
                nc.sync.dma_start(
                    out=spkT[c * 128 : (c + 1) * 128, sl],
                    in_=s_stage[:, sl],
                )

            def load_bias(c):
                if not has_bias:
                    return None
                b_tile = work_pool.tile([128, 1], FP32, tag="bt", name="bt")
                nc.sync.dma_start(out=b_tile, in_=bias[c * 128 : (c + 1) * 128, :])
                return b_tile

            # ---- warmup stream: c0 AND c1 interleaved per k so the PE has
            # ~2.5us of matmul work per arriving X k-tile (the X stream is
            # DMA-bandwidth-paced). W tiles ride in a few big transfers
            # between X blocks; W2 prefetches right after the X bulk.
            WH0, WS0 = alloc_w()
            WH1, WS1 = alloc_w()
            wq = min(8, n_k)
            load_w(0, WH0, WS0, slice(0, wq), conv="dve")
            load_x(slice(0, 2))
            load_w(1, WH1, WS1, slice(0, wq), conv="dve")
            load_x(slice(2, 6))
            load_w(0, WH0, WS0, slice(8, 16), conv="dve")
            load_x(slice(6, 10))
            load_w(1, WH1, WS1, slice(8, 16), conv="dve")
            load_x(slice(10, 14))
            load_w(0, WH0, WS0, slice(16, 24), conv="dve")
            load_x(slice(14, 18))
            load_w(1, WH1, WS1, slice(16, 24), conv="dve")
            load_x(slice(18, 22))
            load_w(0, WH0, WS0, slice(24, n_k), conv="dve")
            load_x(slice(22, 26))
            load_w(1, WH1, WS1, slice(24, n_k), conv="dve")
            for k0 in range(26, n_k, 4):
                load_x(slice(k0, min(k0 + 4, n_k)))
            WH2, WS2 = alloc_w()
            load_w(2, WH2, WS2)

            b0 = load_bias(0)
            b1 = load_bias(1)
            cur0 = work_pool.tile([128, TBl], FP32, tag="cur", name="cur")
            cur1 = work_pool.tile([128, TBl], FP32, tag="cur", name="cur")
            pms0, pcrs0 = alloc_psum()
            pms1, pcrs1 = alloc_psum()
            t_pc = csize // B_loc
            for k in range(n_k):
                for pms, pcrs, WH_c, WS_c in (
                    (pms0, pcrs0, WH0, WS0),
                    (pms1, pcrs1, WH1, WS1),
                ):
                    for ch in range(n_chunk):
                        mm_main(pms[ch], WH_c, k, sls[ch])
                    for ch in range(n_chunk):
                        mm_corr(pcrs[ch], WS_c, k, sls[ch])
            ms0 = alloc_mem()
            ms1 = alloc_mem()
            for ch in range(n_chunk):
                combine(cur0, pms0[ch], pcrs0[ch], sls[ch])
                recurrence(0, cur0, b0, ms0, ch * t_pc, (ch + 1) * t_pc, sls[ch])
                combine(cur1, pms1[ch], pcrs1[ch], sls[ch])
                recurrence(1, cur1, b1, ms1, ch * t_pc, (ch + 1) * t_pc, sls[ch])

            # ---- steady state: chunk-sequential so each chunk's psum stops
            # early and the combine+recurrence overlap the next matmuls.
            # The last TWO c-tiles run as a pair with a joint recurrence
            # (ops on [128,2,B_loc] slices, half the serial chain per column)
            # over shrinking chunks, cutting the post-matmul tail to ~3us.
            steady_end = n_c - 2 if pair_tail else n_c
            for c in range(2, steady_end):
                if c == 2:
                    WH_c, WS_c = WH2, WS2
                else:
                    WH_c, WS_c = alloc_w()
                    load_w(c, WH_c, WS_c)
                b_tile = load_bias(c)
                cur_sb = work_pool.tile([128, TBl], FP32, tag="cur", name="cur")
                ms = alloc_mem()
                if c == n_c - 1 and csize >= 512:
                    bounds = list(range(0, TBl + 1, csize // 2))
                else:
                    bounds = list(range(0, TBl + 1, csize))
                for ch in range(len(bounds) - 1):
                    sl = slice(bounds[ch], bounds[ch + 1])
                    cs2 = bounds[ch + 1] - bounds[ch]
                    pm = pc_pool.tile([128, cs2], FP32, tag="pm", name="pm")
                    pcr = pcc_pool.tile([128, cs2], FP32, tag="pcr", name="pcr")
                    for k in range(n_k):
                        mm_main(pm, WH_c, k, sl)
                    for k in range(n_k):
                        mm_corr(pcr, WS_c, k, sl)
                    combine(cur_sb, pm, pcr, sl)
                    recurrence(
                        c,
                        cur_sb,
                        b_tile,
                        ms,
                        bounds[ch] // B_loc,
                        bounds[ch + 1] // B_loc,
                        sl,
                    )

            if pair_tail:
                cA, cB = n_c - 2, n_c - 1
                WHA, WSA = alloc_w()
                load_w(cA, WHA, WSA)
                WHB, WSB = alloc_w()
                load_w(cB, WHB, WSB)
                cur_p = work_pool.tile(
                    [128, 2, TBl], FP32, tag="curp", name="curp", bufs=1
                )
                s_p = work_pool.tile(
                    [128, 2, TBl], FP16, tag="sp", name="sp", bufs=1
                )
                mem_p = work_pool.tile(
                    [128, 2, B_loc], FP32, tag="memp", name="memp", bufs=1
                )
                nc.vector.memset(mem_p, 0.0)
                bounds = [0, 256, 512, 768, 896, 960, 992, 1024]
                for ch in range(len(bounds) - 1):
                    sl = slice(bounds[ch], bounds[ch + 1])
                    cs2 = bounds[ch + 1] - bounds[ch]
                    pmA = pc_pool.tile([128, cs2], FP32, tag="pm", name="pm")
                    pcrA = pcc_pool.tile([128, cs2], FP32, tag="pcr", name="pcr")
                    pmB = pc_pool.tile([128, cs2], FP32, tag="pm", name="pm")
                    pcrB = pcc_pool.tile([128, cs2], FP32, tag="pcr", name="pcr")
                    for k in range(n_k):
                        mm_main(pmA, WHA, k, sl)
                    for k in range(n_k):
                        mm_corr(pcrA, WSA, k, sl)
                    for k in range(n_k):
                        mm_main(pmB, WHB, k, sl)
                    for k in range(n_k):
                        mm_corr(pcrB, WSB, k, sl)
                    for i, (pm, pcr) in enumerate(((pmA, pcrA), (pmB, pcrB))):
                        nc.scalar.activation(
                            out=cur_p[:, i, sl],
                            in_=pm,
                            func=mybir.ActivationFunctionType.Copy,
                        )
                        nc.vector.scalar_tensor_tensor(
                            out=cur_p[:, i, sl],
                            in0=pcr,
                            scalar=float(2.0**-CORR_E),
                            in1=cur_p[:, i, sl],
                            op0=mybir.AluOpType.mult,
                            op1=mybir.AluOpType.add,
                        )
                    for t in range(bounds[ch] // B_loc, bounds[ch + 1] // B_loc):
                        o = t * B_loc
                        nc.vector.scalar_tensor_tensor(
                            out=mem_p,
                            in0=mem_p,
                            scalar=d,
                            in1=cur_p[:, :, o : o + B_loc],
                            op0=mybir.AluOpType.mult,
                            op1=mybir.AluOpType.add,
                        )
                        s_t = s_p[:, :, o : o + B_loc]
                        nc.vector.tensor_scalar(
                            s_t, mem_p, ths, None, mybir.AluOpType.is_gt
                        )
                        nc.vector.scalar_tensor_tensor(
                            out=mem_p,
                            in0=s_t,
                            scalar=-ths,
                            in1=mem_p,
                            op0=mybir.AluOpType.mult,
                            op1=mybir.AluOpType.add,
                        )
                    # merge the last two tiny chunks' spike DMAs into one to
                    # drop a ~0.7us HWDGE round from the tail
                    if ch == len(bounds) - 3:
                        pass
                    elif ch == len(bounds) - 2:
                        msl = slice(bounds[ch - 1], bounds[ch + 1])
                        nc.sync.dma_start(
                            out=spkT_p[:, :, msl], in_=s_p[:, :, msl]
                        )
                    else:
                        nc.sync.dma_start(out=spkT_p[:, :, sl], in_=s_p[:, :, sl])

    nc.compile()
    return nc


def build_kernel_fp16x3(
    d: float,
    th: float,
    has_bias: bool,
    T=T,
    B_loc=B_LOC,
    CI=CI,
    CO=CO,
):
    """3-pass fp16 hi/lo kernel. All operands arrive from the host already
    split, scaled, and permuted into SBUF tile layout, so the device does
    only matmuls + the recurrence. Spikes leave in [co, tb] layout."""
    TBl = T * B_loc
    n_k = CI // 128
    n_c = CO // 128
    csize = min(512, TBl)
    n_chunk = TBl // csize
    ths = float(th) * SCALE

    nc = bacc.Bacc("TRN2", target_bir_lowering=False, debug=False, num_devices=N_CORES)

    xh = nc.declare_dram_parameter("xh", [128, n_k, TBl], FP16, isOutput=False)
    xl = nc.declare_dram_parameter("xl", [128, n_k, TBl], FP16, isOutput=False)
    wh = nc.declare_dram_parameter("wh", [n_c, 128, n_k, 128], FP16, isOutput=False)
    wl = nc.declare_dram_parameter("wl", [n_c, 128, n_k, 128], FP16, isOutput=False)
    if has_bias:
        bias = nc.declare_dram_parameter("bias", [CO, 1], FP32, isOutput=False)
    spkT = nc.declare_dram_parameter("spkT", [CO, TBl], FP32, isOutput=True)

    with tile.TileContext(nc) as tc:
        with (
            tc.tile_pool(name="xt", bufs=1) as xt_pool,
            tc.tile_pool(name="wt", bufs=2) as wt_pool,
            tc.tile_pool(name="work", bufs=2) as work_pool,
            tc.tile_pool(name="pc", bufs=2 * n_chunk, space="PSUM") as pc_pool,
        ):
            XH = xt_pool.tile([128, n_k, TBl], FP16)
            XL = xt_pool.tile([128, n_k, TBl], FP16)
            # first W strips ahead of the X bulk on the same HWDGE FIFO
            WH_first = wt_pool.tile([128, n_k, 128], FP16, tag="wh")
            WL_first = wt_pool.tile([128, n_k, 128], FP16, tag="wl")
            wq = min(8, n_k)
            for kq in range(0, n_k, wq):
                nc.sync.dma_start(
                    out=WH_first[:, kq : kq + wq, :], in_=wh[0, :, kq : kq + wq, :]
                )
            nc.sync.dma_start(out=WL_first, in_=wl[0, :, :, :])
            for k in range(n_k):
                nc.sync.dma_start(out=XH[:, k, :], in_=xh[:, k, :])
                nc.sync.dma_start(out=XL[:, k, :], in_=xl[:, k, :])

            for c in range(n_c):
                if c == 0:
                    WH_c, WL_c = WH_first, WL_first
                else:
                    WH_c = wt_pool.tile([128, n_k, 128], FP16, tag="wh")
                    WL_c = wt_pool.tile([128, n_k, 128], FP16, tag="wl")
                    nc.sync.dma_start(out=WH_c, in_=wh[c, :, :, :])
                    nc.sync.dma_start(out=WL_c, in_=wl[c, :, :, :])
                if has_bias:
                    b_tile = work_pool.tile([128, 1], FP32, tag="bt")
                    nc.sync.dma_start(
                        out=b_tile, in_=bias[c * 128 : (c + 1) * 128, :]
                    )

                pcs = [
                    pc_pool.tile([128, csize], FP32, tag="pc", name="pc")
                    for _ in range(n_chunk)
                ]
                n_mm = 3 * n_k
                if c == 0:
                    # consume in DMA arrival order: all passes of k before k+1
                    order = [(k, p) for k in range(n_k) for p in (0, 1, 2)]
                else:
                    order = [(k, p) for p in (0, 1, 2) for k in range(n_k)]
                for ch in range(n_chunk):
                    ops = ((WH_c, XH), (WL_c, XH), (WH_c, XL))
                    for i, (k, p) in enumerate(order):
                        Wt, Xt = ops[p]
                        nc.tensor.matmul(
                            pcs[ch],
                            lhsT=Wt[:, k, :],
                            rhs=Xt[:, k, ch * csize : (ch + 1) * csize],
                            start=(i == 0),
                            stop=(i == n_mm - 1),
                        )

                mem = work_pool.tile([128, B_loc], FP32, tag="mem")
                s_stage = work_pool.tile([128, TBl], FP32, tag="s")
                nc.vector.memset(mem, 0.0)
                for t in range(T):
                    o = t * B_loc
                    cur = pcs[o // csize][:, o % csize : o % csize + B_loc]
                    nc.vector.scalar_tensor_tensor(
                        out=mem,
                        in0=mem,
                        scalar=d,
                        in1=cur,
                        op0=mybir.AluOpType.mult,
                        op1=mybir.AluOpType.add,
                    )
                    if has_bias:
                        nc.vector.tensor_scalar(
                            mem, mem, b_tile, None, mybir.AluOpType.add
                        )
                    s_t = s_stage[:, o : o + B_loc]
                    nc.vector.tensor_scalar(
                        s_t, mem, ths, None, mybir.AluOpType.is_gt
                    )
                    nc.vector.scalar_tensor_tensor(
                        out=mem,
                        in0=s_t,
                        scalar=-ths,
                        in1=mem,
                        op0=mybir.AluOpType.mult,
                        op1=mybir.AluOpType.add,
                    )

                nc.sync.dma_start(
                    out=spkT[c * 128 : (c + 1) * 128, :], in_=s_stage
                )

    nc.compile()
    return nc


def build_kernel_fp32hp(
    d: float,
    th: float,
    has_bias: bool,
    T=T,
    B_loc=B_LOC,
    CI=CI,
    CO=CO,
):
    """Exact-fp32 kernel with host-prepped transposed layouts: the device does
    only fp32 matmuls + the recurrence. Spikes leave in [co, tb] layout."""
    TBl = T * B_loc
    n_k = CI // 128
    n_c = CO // 128
    csize = min(512, TBl)
    n_chunk = TBl // csize

    nc = bacc.Bacc("TRN2", target_bir_lowering=False, debug=False, num_devices=N_CORES)

    xt = nc.declare_dram_parameter("xt", [128, n_k, TBl], FP32, isOutput=False)
    wt = nc.declare_dram_parameter("wt", [n_c, 128, n_k, 128], FP32, isOutput=False)
    if has_bias:
        bias = nc.declare_dram_parameter("bias", [CO, 1], FP32, isOutput=False)
    spkT = nc.declare_dram_parameter("spkT", [CO, TBl], FP32, isOutput=True)

    with tile.TileContext(nc) as tc:
        with (
            tc.tile_pool(name="xtp", bufs=1) as xt_pool,
            tc.tile_pool(name="wtp", bufs=3) as wt_pool,
            tc.tile_pool(name="work", bufs=2) as work_pool,
            tc.tile_pool(name="pc", bufs=4 * n_chunk, space="PSUM") as pc_pool,
        ):
            XT = xt_pool.tile([128, n_k, TBl], FP32)
            # first W strip ahead of the XT bulk on the same HWDGE FIFO, in
            # k-chunks, so co-tile 0's first matmuls start almost immediately
            WT_first = wt_pool.tile([128, n_k, 128], FP32, tag="wt")
            wq = min(8, n_k)
            for kq in range(0, n_k, wq):
                nc.sync.dma_start(
                    out=WT_first[:, kq : kq + wq, :], in_=wt[0, :, kq : kq + wq, :]
                )
            # per-k loads so co-tile 0 consumes tiles in DMA arrival order
            for k in range(n_k):
                nc.sync.dma_start(out=XT[:, k, :], in_=xt[:, k, :])

            for c in range(n_c):
                if c == 0:
                    WT_c = WT_first
                else:
                    WT_c = wt_pool.tile([128, n_k, 128], FP32, tag="wt")
                    nc.sync.dma_start(out=WT_c, in_=wt[c, :, :, :])
                if has_bias:
                    b_tile = work_pool.tile([128, 1], FP32, tag="bt")
                    nc.sync.dma_start(
                        out=b_tile, in_=bias[c * 128 : (c + 1) * 128, :]
                    )

                pcs = [
                    pc_pool.tile([128, csize], FP32, tag="pc", name="pc")
                    for _ in range(n_chunk)
                ]
                if c == 0:
                    # k outer: consume XT tiles as they arrive from DRAM
                    for k in range(n_k):
                        for ch in range(n_chunk):
                            nc.tensor.matmul(
                                pcs[ch],
                                lhsT=WT_c[:, k, :],
                                rhs=XT[:, k, ch * csize : (ch + 1) * csize],
                                start=(k == 0),
                                stop=(k == n_k - 1),
                            )
                else:
                    # chunk outer: chunk0 psum frees early for the recurrence
                    for ch in range(n_chunk):
                        for k in range(n_k):
                            nc.tensor.matmul(
                                pcs[ch],
                                lhsT=WT_c[:, k, :],
                                rhs=XT[:, k, ch * csize : (ch + 1) * csize],
                                start=(k == 0),
                                stop=(k == n_k - 1),
                            )

                mem = work_pool.tile([128, B_loc], FP32, tag="mem")
                s_stage = work_pool.tile([128, TBl], FP32, tag="s")
                nc.vector.memset(mem, 0.0)
                for t in range(T):
                    o = t * B_loc
                    cur = pcs[o // csize][:, o % csize : o % csize + B_loc]
                    nc.vector.scalar_tensor_tensor(
                        out=mem,
                        in0=mem,
                        scalar=d,
                        in1=cur,
                        op0=mybir.AluOpType.mult,
                        op1=mybir.AluOpType.add,
                    )
                    if has_bias:
                        nc.vector.tensor_scalar(
                            mem, mem, b_tile, None, mybir.AluOpType.add
                        )
                    s_t = s_stage[:, o : o + B_loc]
                    nc.vector.tensor_scalar(
                        s_t, mem, float(th), None, mybir.AluOpType.is_gt
                    )
                    nc.vector.scalar_tensor_tensor(
                        out=mem,
                        in0=s_t,
                        scalar=-float(th),
                        in1=mem,
                        op0=mybir.AluOpType.mult,
                        op1=mybir.AluOpType.add,
                    )

                nc.sync.dma_start(
                    out=spkT[c * 128 : (c + 1) * 128, :], in_=s_stage
                )

    nc.compile()
    return nc


def _f8(a32, scale):
    """fp32 -> float8_e4m3 at the given power-of-two scale, clipped to range."""
    return np.clip(a32 * np.float32(scale), -240.0, 240.0).astype(
        ml_dtypes.float8_e4m3
    )


def _split16(a32):
    hi = a32.astype(np.float16)
    lo = (a32 - hi.astype(np.float32)).astype(np.float16)
    return hi, lo


def _xt_layout(xs):
    """[TB, CI] -> [128, CI//128, TB] so SBUF partition p holds ci = k*128+p."""
    TBl, CIl = xs.shape
    return np.ascontiguousarray(
        xs.reshape(TBl, CIl // 128, 128).transpose(2, 1, 0)
    )


def _wt_layout(Wm):
    """[CO, CI] -> [CO//128, 128, CI//128, 128]: strip c, partition p=ci%128,
    k=ci//128, j=co%128 -> W[c*128+j, k*128+p]."""
    COl, CIl = Wm.shape
    return np.ascontiguousarray(
        Wm.reshape(COl // 128, 128, CIl // 128, 128).transpose(0, 3, 2, 1)
    )


def kernel(x, W, b, decay, thresh):
    global LAST_EXEC_NS
    x = np.ascontiguousarray(np.asarray(x, dtype=np.float32))
    W = np.ascontiguousarray(np.asarray(W, dtype=np.float32))
    b = np.asarray(b, dtype=np.float32)
    decay = np.asarray(decay, dtype=np.float32)
    thresh = np.asarray(thresh, dtype=np.float32)

    d = float(decay.reshape(-1)[0])
    th = float(thresh.reshape(-1)[0])
    has_bias = bool(np.any(b != 0))

    key = (MODE, d, th, has_bias)
    if key not in _CACHE:
        if MODE == "fp16dr8":
            _CACHE[key] = build_kernel_fp16dr8(d, th, has_bias)
        elif MODE == "fp16x3":
            _CACHE[key] = build_kernel_fp16x3(d, th, has_bias)
        else:
            _CACHE[key] = build_kernel_fp32hp(d, th, has_bias)
    nc = _CACHE[key]

    in_maps = []
    if MODE == "fp16dr8":
        W64 = W * np.float32(WSCALE)
        Wh16 = W64.astype(np.float16)
        Wl = W64 - Wh16.astype(np.float32)
        wh_l = _wt_layout(Wh16)
        wsl_l = _wt_layout(_f8(Wl, WL8_S))
        for i in range(N_CORES):
            xs_i = x[:, i * B_LOC : (i + 1) * B_LOC, :].reshape(TB, CI)
            X128 = xs_i * np.float32(XSCALE)
            Xh16 = X128.astype(np.float16)
            Xl = X128 - Xh16.astype(np.float32)
            m = {
                "xh": _xt_layout(Xh16),
                "xsl": _xt_layout(_f8(Xl, XL8_S)),
                "wh": wh_l,
                "wsl": wsl_l,
            }
            if has_bias:
                m["bias"] = np.ascontiguousarray(
                    (b * np.float32(SCALE)).reshape(CO, 1)
                )
            in_maps.append(m)
    elif MODE == "fp16x3":
        Wh, Wl = _split16(W * np.float32(WSCALE))
        wh_l = _wt_layout(Wh)
        wl_l = _wt_layout(Wl)
        for i in range(N_CORES):
            xs_i = x[:, i * B_LOC : (i + 1) * B_LOC, :].reshape(TB, CI)
            xh_i, xl_i = _split16(xs_i * np.float32(XSCALE))
            m = {
                "xh": _xt_layout(xh_i),
                "xl": _xt_layout(xl_i),
                "wh": wh_l,
                "wl": wl_l,
            }
            if has_bias:
                m["bias"] = np.ascontiguousarray(
                    (b * np.float32(SCALE)).reshape(CO, 1)
                )
            in_maps.append(m)
    else:
        wt_l = _wt_layout(W)
        for i in range(N_CORES):
            xs_i = x[:, i * B_LOC : (i + 1) * B_LOC, :].reshape(TB, CI)
            m = {"xt": _xt_layout(xs_i), "wt": wt_l}
            if has_bias:
                m["bias"] = np.ascontiguousarray(b.reshape(CO, 1))
            in_maps.append(m)

    res = run_bass_kernel_spmd(
        nc, in_maps, core_ids=list(range(N_CORES)), trace=TRACE
    )
    LAST_EXEC_NS = res.exec_time_ns

    # spikes come back [CO, TB]; transpose to [T, B_loc, CO] per core. The
    # paired last two c-tiles return via the fp16 spkT_p plane.
    def unshard(r):
        spk = r["spkT"]
        if "spkT_p" in r:
            spk = spk.copy()
            # spkT_p is [128, 2, TB]: group i holds c-tile (n_c-2+i)'s rows
            spk[CO - 256 : CO - 128, :] = r["spkT_p"][:, 0, :].astype(np.float32)
            spk[CO - 128 :, :] = r["spkT_p"][:, 1, :].astype(np.float32)
        return np.ascontiguousarray(spk.T).reshape(T, B_LOC, CO)

    out = np.concatenate([unshard(r) for r in res.results], axis=1)
    return np.ascontiguousarray(out)

